# revision 1
# baseline (speedup 1.0000x reference)
"""Trainium2 Bass kernel for nn_DAAdj_57114475102829 (GAT-style message passing).

Math (N=4096, F=256, H=8):
  s = x @ Ws.T            [N, H]   (Ws = W_dist[:, :F])
  t'= x @ Wt.T + b_dist   [N, H]   (Wt = W_dist[:, F:])
  z[i,j,h] = s[i,h] + t'[j,h] + (i==j)*selfbias[h]
  heads = softmax(relu(z), axis=j)
  out[i,j] = sum_h heads[i,j,h]*W_merge[0,h] + b_merge[0]

Sharding: rows of i across 8 cores (512 rows each); softmax over j is local.
Each core receives x ROTATED by -c*512 rows so that every core's program is
identical (its own rows are rows 0..512 of its input; the diagonal lands at
j==i in rotated coordinates). The host unrotates output columns.

Per-core device pipeline (per row-block b of 128, per head h):
  E0 = exp(T[j,h] + s[i,h])          scalar engine (bias=per-partition s)
  E0[:, diag] *= exp(selfbias[h])    vector (128x128 only)  [exp(relu(z))=max(exp(z),1)]
  EH = max(E0, 1), denom=sum_j EH    vector tensor_scalar with accum_out
  c = W_merge[h]/denom               vector (per-partition)
  PSUM[:, jc] += diag(c) @ EH_chunk  tensor engine (fp32r), accumulate over h
  out = PSUM + b_merge  -> DMA       vector copy + HWDGE DMA
"""
import sys

sys.path.insert(0, "/opt/trn_rl_repo")

import numpy as np
import concourse.bacc as bacc
from concourse import mybir
from concourse.tile import TileContext
from concourse.bass_utils import run_bass_kernel_spmd

N, F, H = 4096, 256, 8
NCORES = 8
ROWS = N // NCORES  # 512 rows per core
P = 128
NB = ROWS // P      # 4 row blocks per core
JC = 512            # merge chunk = 1 PSUM bank of fp32
NJC = N // JC       # 8 chunks
FP32 = mybir.dt.float32
FP32R = mybir.dt.float32r
AL = mybir.AluOpType
AF = mybir.ActivationFunctionType

# Merge-matmul dtype: fp32 is exact (2-pass hi/lo on the PE); fp32r is fast
# but numerically broken on TRN2 hardware.
MERGE_DT = FP32

_CACHE = {}


def _build():
    nc = bacc.Bacc("TRN2", target_bir_lowering=False, debug=False, num_devices=NCORES)

    x_d = nc.dram_tensor("x", [N, F], FP32, kind="ExternalInput")
    wd_d = nc.dram_tensor("wd", [H, 2 * F], FP32, kind="ExternalInput")
    bd_d = nc.dram_tensor("bd", [H, 1], FP32, kind="ExternalInput")
    wm_d = nc.dram_tensor("wm", [1, H], FP32, kind="ExternalInput")
    bm_d = nc.dram_tensor("bm", [1, 1], FP32, kind="ExternalInput")
    sb_d = nc.dram_tensor("sb", [1, H], FP32, kind="ExternalInput")
    out_d = nc.dram_tensor("out", [ROWS, N], FP32, kind="ExternalOutput")

    with TileContext(nc) as tc:
        with tc.tile_pool(name="persist", bufs=1) as persist:
            # ---- persistent tiles ----
            t_all = persist.tile([P, H, N], FP32, tag="t_all")       # 128 KiB/part
            mask = persist.tile([P, P], FP32, tag="mask")            # identity
            exp_i = persist.tile([P, H, P], FP32, tag="expi")        # diag exp(sb)
            s_all = persist.tile([P, NB, H], FP32, tag="s_all")
            wm_b = persist.tile([P, H], FP32, tag="wm_b")
            bm_c = persist.tile([P, 1], FP32, tag="bm_c")
            sb_b = persist.tile([P, H], FP32, tag="sb_b")
            bd_c = persist.tile([H, 1], FP32, tag="bd_c")
            it_p = persist.tile([P, 1], FP32, tag="it_p")
            it_f = persist.tile([P, P], FP32, tag="it_f")

            with tc.tile_pool(name="dram", bufs=1, space="DRAM") as dpool:
                tpd = dpool.tile([H, N], FP32)

                # ================= startup =================
                # broadcast small params across partitions (stride-0 DMA)
                nc.sync.dma_start(out=wm_b, in_=wm_d[0:1, :].to_broadcast((P, H)))
                nc.sync.dma_start(out=bm_c, in_=bm_d[0:1, :].to_broadcast((P, 1)))
                nc.sync.dma_start(out=sb_b, in_=sb_d[0:1, :].to_broadcast((P, H)))
                nc.sync.dma_start(out=bd_c, in_=bd_d[:, :])

                # identity mask via iota + compare
                nc.gpsimd.iota(
                    it_p, [[0, 1]], channel_multiplier=1,
                    allow_small_or_imprecise_dtypes=True,
                )
                nc.gpsimd.iota(
                    it_f, [[1, P]], channel_multiplier=0,
                    allow_small_or_imprecise_dtypes=True,
                )
                nc.vector.tensor_scalar(mask, it_f, it_p[:, 0:1], None, AL.is_equal)

                # exp_i[h] = 1 + mask*(exp(sb[h]) - 1)
                esb = persist.tile([P, H], FP32, tag="esb")
                nc.scalar.activation(esb, sb_b, AF.Exp)
                nc.vector.tensor_scalar(esb, esb, -1.0, None, AL.add)
                for h in range(H):
                    nc.vector.tensor_scalar(
                        exp_i[:, h, :], mask, esb[:, h : h + 1], 1.0, AL.mult, AL.add
                    )

                with (
                    tc.tile_pool(name="su1", bufs=1) as su1,
                    tc.tile_pool(name="su", bufs=2) as su,
                ):
                    # W transposes via strided DMA (tiny):
                    # wst[fh] = Ws.T[fh*128:(fh+1)*128, :], wtt likewise for Wt
                    wst = []
                    wtt = []
                    for fh in range(2):
                        wsts = su.tile([P, H], FP32, tag=f"wst{fh}")
                        nc.sync.dma_start(
                            out=wsts,
                            in_=wd_d[0:H, fh * P : (fh + 1) * P].transpose([1, 0]),
                        )
                        wst.append(wsts)
                        wtts = su.tile([P, H], FP32, tag=f"wtt{fh}")
                        nc.sync.dma_start(
                            out=wtts,
                            in_=wd_d[0:H, F + fh * P : F + (fh + 1) * P].transpose(
                                [1, 0]
                            ),
                        )
                        wtt.append(wtts)

                    # x.T [256, 4096] as two [128, 4096] tiles, via PE transposes
                    xt = [
                        su1.tile([P, N], FP32, tag=f"xt{fh}", name=f"xt{fh}")
                        for fh in range(2)
                    ]
                    with tc.tile_pool(name="ps_tr", bufs=4, space="PSUM") as ps_tr:
                        for rt in range(N // ROWS):  # 8 groups of 4 row-tiles
                            xbig = su.tile([P, NB, F], FP32, tag="xbig")
                            nc.sync.dma_start(
                                out=xbig,
                                in_=x_d[rt * ROWS : (rt + 1) * ROWS, :].rearrange(
                                    "(a p) f -> p a f", p=P
                                ),
                            )
                            for a in range(NB):
                                col = rt * ROWS + a * P
                                for fh in range(2):
                                    pst = ps_tr.tile([P, P], FP32, tag="tr")
                                    nc.tensor.transpose(
                                        pst, xbig[:, a, fh * P : (fh + 1) * P], mask
                                    )
                                    eng = nc.vector if (a + fh) % 2 == 0 else nc.scalar
                                    if eng is nc.vector:
                                        nc.vector.tensor_copy(
                                            xt[fh][:, col : col + P], pst
                                        )
                                    else:
                                        nc.scalar.copy(xt[fh][:, col : col + P], pst)

                    with tc.tile_pool(name="ps_stp", bufs=2, space="PSUM") as ps_stp:
                        # s_all[i, b, h] for this core's rows (= cols 0..512 of x.T)
                        for b in range(NB):
                            ps_s = ps_stp.tile([P, H], FP32, tag="s")
                            nc.tensor.matmul(
                                ps_s,
                                lhsT=xt[0][:, b * P : (b + 1) * P],
                                rhs=wst[0],
                                start=True,
                                stop=False,
                            )
                            nc.tensor.matmul(
                                ps_s,
                                lhsT=xt[1][:, b * P : (b + 1) * P],
                                rhs=wst[1],
                                start=False,
                                stop=True,
                            )
                            nc.vector.tensor_copy(s_all[:, b, :], ps_s)

                        # t'_T [8, 4096] = Wt @ x.T + b_dist
                        tp_t = su1.tile([H, N], FP32, tag="tp_t")
                        for jc in range(NJC):
                            ps_t = ps_stp.tile([H, JC], FP32, tag="t")
                            nc.tensor.matmul(
                                ps_t,
                                lhsT=wtt[0],
                                rhs=xt[0][:, jc * JC : (jc + 1) * JC],
                                start=True,
                                stop=False,
                            )
                            nc.tensor.matmul(
                                ps_t,
                                lhsT=wtt[1],
                                rhs=xt[1][:, jc * JC : (jc + 1) * JC],
                                start=False,
                                stop=True,
                            )
                            nc.scalar.activation(
                                tp_t[:, jc * JC : (jc + 1) * JC],
                                ps_t,
                                AF.Identity,
                                bias=bd_c[:, 0:1],
                            )

                        # round-trip t' through HBM, broadcast to 128 partitions
                        nc.sync.dma_start(out=tpd, in_=tp_t)
                        for h in range(H):
                            nc.sync.dma_start(
                                out=t_all[:, h, :],
                                in_=tpd[h : h + 1, :].to_broadcast((P, N)),
                            )

                # ================= steady state =================
                with (
                    tc.tile_pool(name="big", bufs=3) as big,
                    tc.tile_pool(name="dcp", bufs=2) as dcp,
                    tc.tile_pool(name="small", bufs=6) as small,
                    tc.tile_pool(name="ost", bufs=2) as ost,
                    tc.tile_pool(name="mps", bufs=1, space="PSUM") as mps,
                ):
                    for b in range(NB):
                        dr = b * P  # diagonal column range start
                        psum_tiles = [
                            mps.tile([P, JC], FP32, tag=f"m{jc}", name=f"m{b}_{jc}")
                            for jc in range(NJC)
                        ]
                        for h in range(H):
                            e0 = big.tile([P, N], FP32, tag="big")
                            nc.scalar.activation(
                                e0,
                                t_all[:, h, :],
                                AF.Exp,
                                bias=s_all[:, b, h : h + 1],
                            )
                            # diagonal selfbias fix (only i==j block columns)
                            nc.vector.tensor_tensor(
                                out=e0[:, dr : dr + P],
                                in0=e0[:, dr : dr + P],
                                in1=exp_i[:, h, :],
                                op=AL.mult,
                            )
                            eh = big.tile([P, N], MERGE_DT, tag="big")
                            denom = small.tile([P, 1], FP32, tag="denom")
                            nc.vector.tensor_scalar(
                                eh, e0, 1.0, None, AL.max, AL.add, accum_out=denom
                            )
                            recip = small.tile([P, 1], FP32, tag="recip")
                            nc.vector.reciprocal(recip, denom)
                            cvec = small.tile([P, 1], FP32, tag="cvec")
                            nc.vector.tensor_scalar(
                                cvec, recip, wm_b[:, h : h + 1], None, AL.mult
                            )
                            dc = dcp.tile([P, P], MERGE_DT, tag="dc")
                            nc.vector.tensor_scalar(
                                dc, mask, cvec[:, 0:1], None, AL.mult
                            )
                            for jc in range(NJC):
                                nc.tensor.matmul(
                                    psum_tiles[jc],
                                    lhsT=dc,
                                    rhs=eh[:, jc * JC : (jc + 1) * JC],
                                    start=(h == 0),
                                    stop=(h == H - 1),
                                )
                        # drain block: PSUM -> SBUF (+b_merge) -> HBM
                        for jh in range(2):
                            o = ost.tile([P, N // 2], FP32, tag="ost")
                            for q in range(NJC // 2):
                                jc = jh * (NJC // 2) + q
                                nc.vector.tensor_scalar(
                                    o[:, q * JC : (q + 1) * JC],
                                    psum_tiles[jc],
                                    bm_c[:, 0:1],
                                    None,
                                    AL.add,
                                )
                            nc.sync.dma_start(
                                out=out_d[
                                    b * P : (b + 1) * P,
                                    jh * (N // 2) : (jh + 1) * (N // 2),
                                ],
                                in_=o,
                            )

    nc.compile()
    return nc


def _get_nc():
    if "nc" not in _CACHE:
        _CACHE["nc"] = _build()
    return _CACHE["nc"]


def _in_maps(inputs):
    x = np.ascontiguousarray(np.asarray(inputs["x"], dtype=np.float32))
    W_dist = np.ascontiguousarray(np.asarray(inputs["W_dist"], dtype=np.float32))
    b_dist = np.asarray(inputs["b_dist"], dtype=np.float32).reshape(H, 1)
    W_merge = np.asarray(inputs["W_merge"], dtype=np.float32).reshape(1, H)
    b_merge = np.asarray(inputs["b_merge"], dtype=np.float32).reshape(1, 1)
    selfbias = np.asarray(inputs["selfbias"], dtype=np.float32).reshape(1, H)
    in_maps = []
    for c in range(NCORES):
        in_maps.append(
            {
                "x": np.ascontiguousarray(np.roll(x, -c * ROWS, axis=0)),
                "wd": W_dist,
                "bd": b_dist,
                "wm": W_merge,
                "bm": b_merge,
                "sb": selfbias,
            }
        )
    return in_maps


def _assemble(results):
    out = np.empty((N, N), dtype=np.float32)
    for c in range(NCORES):
        out[c * ROWS : (c + 1) * ROWS, :] = np.roll(
            results[c]["out"], c * ROWS, axis=1
        )
    return out


def kernel(x, W_dist, b_dist, W_merge, b_merge, selfbias):
    nc = _get_nc()
    in_maps = _in_maps(
        {
            "x": x,
            "W_dist": W_dist,
            "b_dist": b_dist,
            "W_merge": W_merge,
            "b_merge": b_merge,
            "selfbias": selfbias,
        }
    )
    res = run_bass_kernel_spmd(nc, in_maps, core_ids=list(range(NCORES)))
    return _assemble(res.results)



# revision 3
# speedup vs baseline: 1.7663x; 1.7663x over previous
"""Trainium2 Bass kernel for nn_DAAdj_57114475102829 (GAT-style message passing).

Math (N=4096, F=256, H=8):
  s = x @ Ws.T            [N, H]   (Ws = W_dist[:, :F])
  t'= x @ Wt.T + b_dist   [N, H]   (Wt = W_dist[:, F:])
  z[i,j,h] = s[i,h] + t'[j,h] + (i==j)*selfbias[h]
  heads = softmax(relu(z), axis=j)
  out[i,j] = sum_h heads[i,j,h]*W_merge[0,h] + b_merge[0]

Key identity: exp(relu(z)) = max(exp(z), 1) = 1 + relu(exp(z) - 1), with
exp(z) = a_ih * e_jh for a = exp(s), e = exp(t').  So the whole N^2*H
elementwise phase is ONE fused instruction per (row-block, head):
  Act engine:  G  = Relu(a*E - 1)   (accum_out gives Z - 4096)
  DVE engine:  EH = max(a*E, 1)     (scalar_tensor_tensor, accum_out = Z)
and the merge sum_h c_ih * EH is a bf16 PE matmul with stationary
diag(c), c = W_merge[h]/Z. The +1 offset of G-heads folds into the
drain bias K = b_merge + sum_{G-heads} c_h.

Diagonal selfbias: handled as per-row corrections (no N^2 work):
  delta_denom = max(u*e^sb,1) - max(u,1), u = exp(s_i + t'_i)
  out[i,i]   += sum_h c_h*delta_h  -- via one extra PE matmul against a
  shifted-identity tile (M896 view).

Sharding: rows of i across 8 cores (512 rows each); x ROTATED by -c*512
rows per core so the diagonal lands at cols b*128..(b+1)*128 of block b.
"""
import sys

sys.path.insert(0, "/opt/trn_rl_repo")

import numpy as np
import concourse.bacc as bacc
from concourse import mybir
from concourse.tile import TileContext
from concourse.bass_utils import run_bass_kernel_spmd

N, F, H = 4096, 256, 8
NCORES = 8
ROWS = N // NCORES  # 512 rows per core
P = 128
NB = ROWS // P      # 4 row blocks per core
JC = 512            # PSUM bank chunk (fp32)
NJC = N // JC       # 8 chunks
FP32 = mybir.dt.float32
BF16 = mybir.dt.bfloat16
AL = mybir.AluOpType
AF = mybir.ActivationFunctionType

# heads produced on the scalar (Act) engine per block; rest on DVE
ACT_N = [4, 4, 4, 4]

_CACHE = {}


def _build():
    nc = bacc.Bacc("TRN2", target_bir_lowering=False, debug=False, num_devices=NCORES)

    x_d = nc.dram_tensor("x", [N, F], BF16, kind="ExternalInput")
    wd_d = nc.dram_tensor("wd", [H, 2 * F], BF16, kind="ExternalInput")
    bd_d = nc.dram_tensor("bd", [H, 1], FP32, kind="ExternalInput")
    bdr_d = nc.dram_tensor("bdr", [1, H], FP32, kind="ExternalInput")
    wm_d = nc.dram_tensor("wm", [1, H], FP32, kind="ExternalInput")
    bm_d = nc.dram_tensor("bm", [1, 1], FP32, kind="ExternalInput")
    sb_d = nc.dram_tensor("sb", [1, H], FP32, kind="ExternalInput")
    out_d = nc.dram_tensor("out", [ROWS, N], FP32, kind="ExternalOutput")

    with TileContext(nc) as tc:
        with tc.tile_pool(name="persist", bufs=1) as persist:
            # ---- persistent tiles ----
            e_all = persist.tile([P, H, N], BF16, tag="e_all")   # 64 KiB/part
            ones = persist.tile([P, N], BF16, tag="ones")
            mask = persist.tile([P, P], FP32, tag="mask")
            maskbf = persist.tile([P, P], BF16, tag="maskbf")
            m896 = persist.tile([P, 896], BF16, tag="m896")
            maskw = persist.tile([P, H, P], BF16, tag="maskw")
            a_all = persist.tile([P, NB, H], FP32, tag="a_all")
            dd8 = persist.tile([P, NB, H], FP32, tag="dd8")
            del8 = persist.tile([P, NB, H], FP32, tag="del8")
            wm_b = persist.tile([P, H], FP32, tag="wm_b")
            bm_c = persist.tile([P, 1], FP32, tag="bm_c")
            sb_b = persist.tile([P, H], FP32, tag="sb_b")
            bdr_b = persist.tile([P, H], FP32, tag="bdr_b")
            bd_c = persist.tile([H, 1], FP32, tag="bd_c")
            neg1 = persist.tile([P, 1], FP32, tag="neg1")
            esb = persist.tile([P, H], FP32, tag="esb")
            it_p = persist.tile([P, 1], FP32, tag="it_p")
            it_f = persist.tile([P, P], FP32, tag="it_f")

            with tc.tile_pool(name="dram", bufs=1, space="DRAM") as dpool:
                edd = dpool.tile([H, N], BF16)

                # ================= startup =================
                nc.sync.dma_start(out=wm_b, in_=wm_d[0:1, :].to_broadcast((P, H)))
                nc.sync.dma_start(out=bm_c, in_=bm_d[0:1, :].to_broadcast((P, 1)))
                nc.sync.dma_start(out=sb_b, in_=sb_d[0:1, :].to_broadcast((P, H)))
                nc.sync.dma_start(out=bdr_b, in_=bdr_d[0:1, :].to_broadcast((P, H)))
                nc.sync.dma_start(out=bd_c, in_=bd_d[:, :])

                nc.vector.memset(neg1, -1.0)
                nc.vector.memset(ones, 1.0)
                nc.gpsimd.iota(
                    it_p, [[0, 1]], channel_multiplier=1,
                    allow_small_or_imprecise_dtypes=True,
                )
                nc.gpsimd.iota(
                    it_f, [[1, P]], channel_multiplier=0,
                    allow_small_or_imprecise_dtypes=True,
                )
                nc.vector.tensor_scalar(mask, it_f, it_p[:, 0:1], None, AL.is_equal)
                nc.vector.tensor_copy(maskbf, mask)
                nc.vector.memset(m896, 0.0)
                nc.vector.tensor_copy(m896[:, 384:512], maskbf)
                nc.scalar.activation(esb, sb_b, AF.Exp)
                for h in range(H):
                    nc.vector.tensor_scalar(
                        maskw[:, h, :], mask, wm_b[:, h : h + 1], None, AL.mult
                    )

                with (
                    tc.tile_pool(name="su1", bufs=1) as su1,
                    tc.tile_pool(name="su2", bufs=2) as su2,
                    tc.tile_pool(name="ps_su", bufs=2, space="PSUM") as ps_su,
                ):
                    # x.T via DMA xbar transpose (bf16): [256, 4096] as 2 tiles
                    xt = []
                    for fh in range(2):
                        xts = su1.tile([P, N], BF16, tag=f"xt{fh}")
                        nc.sync.dma_start_transpose(
                            out=xts, in_=x_d[:, fh * P : (fh + 1) * P]
                        )
                        xt.append(xts)

                    # W transposes: wstt[fh] [128, 2H] = [Ws.T | Wt.T] block fh
                    wstt = []
                    for fh in range(2):
                        w = su1.tile([P, 2 * H], BF16, tag=f"wstt{fh}")
                        nc.sync.dma_start(
                            out=w[:, 0:H],
                            in_=wd_d[0:H, fh * P : (fh + 1) * P].transpose([1, 0]),
                        )
                        nc.sync.dma_start(
                            out=w[:, H : 2 * H],
                            in_=wd_d[0:H, F + fh * P : F + (fh + 1) * P].transpose(
                                [1, 0]
                            ),
                        )
                        wstt.append(w)

                    # per-block: s, t'_own -> a_all, del8, dd8
                    for b in range(NB):
                        ps8 = ps_su.tile([P, 2 * H], FP32, tag="ps8")
                        nc.tensor.matmul(
                            ps8,
                            lhsT=xt[0][:, b * P : (b + 1) * P],
                            rhs=wstt[0],
                            start=True,
                            stop=False,
                        )
                        nc.tensor.matmul(
                            ps8,
                            lhsT=xt[1][:, b * P : (b + 1) * P],
                            rhs=wstt[1],
                            start=False,
                            stop=True,
                        )
                        s16 = su2.tile([P, 2 * H], FP32, tag="s16")
                        nc.vector.tensor_copy(s16, ps8)
                        nc.scalar.activation(a_all[:, b, :], s16[:, 0:H], AF.Exp)
                        ts = su2.tile([P, H], FP32, tag="ts")
                        nc.vector.tensor_tensor(
                            out=ts, in0=s16[:, 0:H], in1=s16[:, H : 2 * H], op=AL.add
                        )
                        v = su2.tile([P, H], FP32, tag="v")
                        nc.vector.tensor_tensor(out=v, in0=ts, in1=bdr_b, op=AL.add)
                        u = su2.tile([P, H], FP32, tag="u")
                        nc.scalar.activation(u, v, AF.Exp)
                        ue = su2.tile([P, H], FP32, tag="ue")
                        nc.vector.tensor_tensor(out=ue, in0=u, in1=esb, op=AL.mult)
                        m1 = su2.tile([P, H], FP32, tag="m1")
                        nc.vector.tensor_scalar(m1, u, 1.0, None, AL.max)
                        m2 = su2.tile([P, H], FP32, tag="m2")
                        nc.vector.tensor_scalar(m2, ue, 1.0, None, AL.max)
                        nc.vector.tensor_tensor(
                            out=del8[:, b, :], in0=m2, in1=m1, op=AL.subtract
                        )
                        na = ACT_N[b]
                        nc.vector.tensor_scalar(
                            dd8[:, b, 0:na], del8[:, b, 0:na], float(N), None, AL.add
                        )
                        if na < H:
                            nc.vector.tensor_copy(
                                dd8[:, b, na:H], del8[:, b, na:H]
                            )

                    # t'.T -> exp -> Ed (bf16) -> DRAM -> broadcast
                    ed = su1.tile([H, N], BF16, tag="ed")
                    for jc in range(NJC):
                        ps_t = ps_su.tile([H, JC], FP32, tag="ps_t")
                        nc.tensor.matmul(
                            ps_t,
                            lhsT=wstt[0][:, H : 2 * H],
                            rhs=xt[0][:, jc * JC : (jc + 1) * JC],
                            start=True,
                            stop=False,
                        )
                        nc.tensor.matmul(
                            ps_t,
                            lhsT=wstt[1][:, H : 2 * H],
                            rhs=xt[1][:, jc * JC : (jc + 1) * JC],
                            start=False,
                            stop=True,
                        )
                        nc.scalar.activation(
                            ed[:, jc * JC : (jc + 1) * JC],
                            ps_t,
                            AF.Exp,
                            bias=bd_c[:, 0:1],
                        )
                    nc.sync.dma_start(out=edd, in_=ed)
                    for h in range(H):
                        nc.sync.dma_start(
                            out=e_all[:, h, :],
                            in_=edd[h : h + 1, :].to_broadcast((P, N)),
                        )

                # ================= steady state =================
                with (
                    tc.tile_pool(name="big", bufs=3) as big,
                    tc.tile_pool(name="dcp", bufs=3) as dcp,
                    tc.tile_pool(name="small", bufs=2) as small,
                    tc.tile_pool(name="ost", bufs=2) as ost,
                    tc.tile_pool(name="mps", bufs=1, space="PSUM") as mps,
                ):
                    for b in range(NB):
                        na = ACT_N[b]
                        psum = mps.tile([P, N], FP32, tag="psum", name=f"psum{b}")
                        acc8 = small.tile([P, H], FP32, tag="acc8")
                        z8 = small.tile([P, H], FP32, tag="z8")
                        r8 = small.tile([P, H], FP32, tag="r8")

                        # interleave Act-heads (0..na-1) and DVE-heads (na..7)
                        order = []
                        ai, di = 0, na
                        for k in range(H):
                            if k % 2 == 0 and ai < na:
                                order.append(ai)
                                ai += 1
                            elif di < H:
                                order.append(di)
                                di += 1
                            else:
                                order.append(ai)
                                ai += 1

                        for idx, h in enumerate(order):
                            eh = big.tile([P, N], BF16, tag="eh")
                            if h < na:
                                # G = relu(a*E - 1), acc = sum G = Z - N
                                nc.scalar.activation(
                                    eh,
                                    e_all[:, h, :],
                                    AF.Relu,
                                    bias=neg1[:, 0:1],
                                    scale=a_all[:, b, h : h + 1],
                                    accum_out=acc8[:, h : h + 1],
                                )
                            else:
                                # EH = max(a*E, 1), acc = Z
                                nc.vector.scalar_tensor_tensor(
                                    eh,
                                    e_all[:, h, :],
                                    a_all[:, b, h : h + 1],
                                    ones,
                                    AL.mult,
                                    AL.max,
                                    accum_out=acc8[:, h : h + 1],
                                )
                            nc.vector.tensor_tensor(
                                out=z8[:, h : h + 1],
                                in0=acc8[:, h : h + 1],
                                in1=dd8[:, b, h : h + 1],
                                op=AL.add,
                            )
                            nc.vector.reciprocal(r8[:, h : h + 1], z8[:, h : h + 1])
                            dc = dcp.tile([P, P], BF16, tag="dc")
                            nc.vector.tensor_scalar(
                                dc, maskw[:, h, :], r8[:, h : h + 1], None, AL.mult
                            )
                            for jc in range(NJC):
                                nc.tensor.matmul(
                                    psum[:, jc * JC : (jc + 1) * JC],
                                    lhsT=dc,
                                    rhs=eh[:, jc * JC : (jc + 1) * JC],
                                    start=(idx == 0),
                                    stop=(idx == H - 1 and jc > 0),
                                )

                        # diagonal fix + drain
                        c8 = small.tile([P, H], FP32, tag="c8")
                        nc.vector.tensor_tensor(out=c8, in0=r8, in1=wm_b, op=AL.mult)
                        t8 = small.tile([P, H], FP32, tag="t8")
                        nc.vector.tensor_tensor(
                            out=t8, in0=c8, in1=del8[:, b, :], op=AL.mult
                        )
                        dlt = small.tile([P, 1], FP32, tag="dlt")
                        nc.vector.tensor_reduce(
                            dlt, t8, axis=mybir.AxisListType.X, op=AL.add
                        )
                        k1 = small.tile([P, 1], FP32, tag="k1")
                        nc.vector.tensor_reduce(
                            k1, c8[:, 0:na], axis=mybir.AxisListType.X, op=AL.add
                        )
                        kb = small.tile([P, 1], FP32, tag="kb")
                        nc.vector.tensor_tensor(out=kb, in0=k1, in1=bm_c, op=AL.add)
                        dcd = dcp.tile([P, P], BF16, tag="dc")
                        nc.vector.tensor_scalar(
                            dcd, maskbf, dlt[:, 0:1], None, AL.mult
                        )
                        nc.tensor.matmul(
                            psum[:, 0:JC],
                            lhsT=dcd,
                            rhs=m896[:, 384 - P * b : 896 - P * b],
                            start=False,
                            stop=True,
                        )

                        stage = ost.tile([P, N], FP32, tag="stage")
                        nc.scalar.activation(
                            stage, psum, AF.Identity, bias=kb[:, 0:1]
                        )
                        nc.sync.dma_start(
                            out=out_d[b * P : (b + 1) * P, :], in_=stage
                        )

    nc.compile()
    return nc


def _get_nc():
    if "nc" not in _CACHE:
        _CACHE["nc"] = _build()
    return _CACHE["nc"]


def _in_maps(inputs):
    import ml_dtypes

    x = np.ascontiguousarray(np.asarray(inputs["x"], dtype=np.float32))
    W_dist = np.asarray(inputs["W_dist"], dtype=np.float32).astype(ml_dtypes.bfloat16)
    b_dist = np.asarray(inputs["b_dist"], dtype=np.float32).reshape(H, 1)
    W_merge = np.asarray(inputs["W_merge"], dtype=np.float32).reshape(1, H)
    b_merge = np.asarray(inputs["b_merge"], dtype=np.float32).reshape(1, 1)
    selfbias = np.asarray(inputs["selfbias"], dtype=np.float32).reshape(1, H)
    in_maps = []
    for c in range(NCORES):
        xr = np.roll(x, -c * ROWS, axis=0).astype(ml_dtypes.bfloat16)
        in_maps.append(
            {
                "x": np.ascontiguousarray(xr),
                "wd": W_dist,
                "bd": b_dist,
                "bdr": np.ascontiguousarray(b_dist.reshape(1, H)),
                "wm": W_merge,
                "bm": b_merge,
                "sb": selfbias,
            }
        )
    return in_maps


def _assemble(results):
    out = np.empty((N, N), dtype=np.float32)
    for c in range(NCORES):
        out[c * ROWS : (c + 1) * ROWS, :] = np.roll(
            results[c]["out"], c * ROWS, axis=1
        )
    return out


def kernel(x, W_dist, b_dist, W_merge, b_merge, selfbias):
    nc = _get_nc()
    in_maps = _in_maps(
        {
            "x": x,
            "W_dist": W_dist,
            "b_dist": b_dist,
            "W_merge": W_merge,
            "b_merge": b_merge,
            "selfbias": selfbias,
        }
    )
    res = run_bass_kernel_spmd(nc, in_maps, core_ids=list(range(NCORES)))
    return _assemble(res.results)
